# revision 37
# baseline (speedup 1.0000x reference)
"""DurationConditioningProjector Trainium2 kernel.

Data-parallel over batch B=16 across 8 NeuronCores (2 items per core).
Everything is computed on-device; the host only slices/replicates inputs
(pure relayout of weight matrices) and reassembles the output.

Per-item layout: residual x as (C=128 partitions, T free) fp32 in SBUF.
- Length-regulator upsample AS MATMUL: idx is monotone (cumsum of
  durations), so x[:,t] = P''[idx[t]] is computed as paired band-mask
  matmuls  sum_m P''[m]*[t>=cum[m-1]] - P''[m]*[t>=cum[m]]  where the
  {0,1} masks come from DVE is_ge tensor_scalar ops on an iota tile.
  The two mask terms cancel exactly off-band, so each frame nets exactly
  one bf16 P'' row (gather-grade precision; no dma_gather, no DRAM
  index/table round-trips). in_b+pos_b ride in as a rank-1 ones matmul;
  the sin/cos pos-emb matmul accumulates into the same PSUM bank.
- LayerNorm stays on-chip: per-slab channel sums/sumsq accumulate into
  (8,F) PSUM rows via one-hot-column matmuls; row math (mean/var, rsqrt
  via Sqrt + reciprocal_approx_fast) on (8,F) tiles; per-frame scale and
  offset broadcast to (128,F) by 0-stride DMA from a small DRAM bounce;
  normalize = 2 DVE TTs + fused ACT gelu(g*z+b).
- 3 dilated causal conv layers: 31 shifted bf16 matmuls per slab into
  PSUM; residual add is one fused scalar_tensor_tensor. Emission is
  software-pipelined at slab granularity (norm slab -> conv slab, other
  item's stats injected mid-stream) so the PE never drains and HAM
  stays warm. Output LN + per-chunk matmuls with out_b folded in as a
  rank-1 accumulate.
"""
import sys
sys.path.insert(0, '/opt/trn_rl_repo')

import math
import os
import numpy as np

import concourse.bass as bass
import concourse.mybir as mybir
import concourse.tile as tile
from concourse import bacc
from concourse import bass_utils

dt = mybir.dt
Alu = mybir.AluOpType
ActF = mybir.ActivationFunctionType
_GELU = ActF.Tanh if os.environ.get('KSIM_TANH') else ActF.Gelu

B, N, DIN, C, DOUT, K, L = 16, 1024, 256, 128, 256, 31, 3
NCORES = 8
BPC = B // NCORES
TWO_PI = 2.0 * math.pi
EPS = 1e-5


def _ceil_to(x, m):
    return (x + m - 1) // m * m


def build_nc(T):
    TP = _ceil_to(T, 128)
    NT = TP // 128
    F = TP // 8
    assert F % 16 == 0 and F <= 512
    NCH = N // 128
    NGC = (TP + 511) // 512       # gather chunks

    nc = bacc.Bacc("TRN2", target_bir_lowering=False, debug=False)

    pooled = nc.dram_tensor("pooled", [BPC, N, DIN], dt.float32, kind="ExternalInput").ap()
    durations = nc.dram_tensor("durations", [BPC, N], dt.int32, kind="ExternalInput").ap()
    rel_pos = nc.dram_tensor("rel_pos", [BPC, T], dt.float32, kind="ExternalInput").ap()
    in_wT = nc.dram_tensor("in_wT", [DIN, C], dt.float32, kind="ExternalInput").ap()
    in_b = nc.dram_tensor("in_b", [C], dt.float32, kind="ExternalInput").ap()
    pos_wT = nc.dram_tensor("pos_wT", [C, C], dt.float32, kind="ExternalInput").ap()
    pos_b = nc.dram_tensor("pos_b", [C], dt.float32, kind="ExternalInput").ap()
    conv_wr = nc.dram_tensor("conv_wr", [L, K, C, C], dt.float32, kind="ExternalInput").ap()
    conv_b = nc.dram_tensor("conv_b", [L, C], dt.float32, kind="ExternalInput").ap()
    ln_g = nc.dram_tensor("ln_g", [L, C], dt.float32, kind="ExternalInput").ap()
    ln_b = nc.dram_tensor("ln_b", [L, C], dt.float32, kind="ExternalInput").ap()
    out_ln_g = nc.dram_tensor("out_ln_g", [C], dt.float32, kind="ExternalInput").ap()
    out_ln_b = nc.dram_tensor("out_ln_b", [C], dt.float32, kind="ExternalInput").ap()
    out_wT = nc.dram_tensor("out_wT", [C, DOUT], dt.float32, kind="ExternalInput").ap()
    out_b = nc.dram_tensor("out_b", [DOUT], dt.float32, kind="ExternalInput").ap()
    ipb_row = nc.dram_tensor("ipb_row", [1, C], dt.float32, kind="ExternalInput").ap()
    out = nc.dram_tensor("out", [BPC, T, DOUT], dt.float32, kind="ExternalOutput").ap()

    ident_c = nc.inline_tensor(np.eye(128, dtype=np.float32), "identc")
    iota_c = nc.inline_tensor(
        np.broadcast_to(np.arange(F, dtype=np.float32), (128, F)).copy(), "iotac")
    # onehot_c[p, si, j] = 1 iff j == si : lhsT column-selector for stats rows
    oh = np.zeros((8, 8), np.float32)
    np.fill_diagonal(oh, 1.0)
    onehot_c = nc.inline_tensor(
        np.broadcast_to(oh[None, :, :], (128, 8, 8)).copy(), "onehotc")
    half = C // 2
    freqs = np.exp(-math.log(10000.0) * np.arange(half, dtype=np.float64) / (half - 1))
    fc2pi_c = nc.inline_tensor(
        (np.concatenate([freqs, freqs]) / TWO_PI).astype(np.float32)[:, None], "fc2pic")
    coff_np = np.zeros((C, 1), np.float32)
    coff_np[half:] = 0.25
    coff_c = nc.inline_tensor(coff_np, "coffc")

    dils = [2 ** i for i in range(L)]
    HPAD = (K - 1) * dils[-1]

    with tile.TileContext(nc) as tc:
        import contextlib
        ctx = contextlib.ExitStack()
        with ctx:
            pers = ctx.enter_context(tc.tile_pool(name="pers", bufs=1))
            pp = ctx.enter_context(tc.tile_pool(name="pp", bufs=2))
            psum = ctx.enter_context(tc.tile_pool(name="ps", bufs=1, space="PSUM"))
            dram = ctx.enter_context(tc.tile_pool(name="dr", bufs=1, space="DRAM"))

            def ptile(shape, d, nm, bufs=1, pool=None):
                pool = pool or pers
                return pool.tile(shape, d, tag=nm, name=nm, bufs=bufs)

            # ---------- shared setup ----------
            ident = ptile([128, 128], dt.float32, "ident")
            nc.sync.dma_start(out=ident, in_=ident_c.ap())
            iot = ptile([128, F], dt.float32, "iot")
            nc.sync.dma_start(out=iot, in_=iota_c.ap())
            fc2pi = ptile([C, 1], dt.float32, "fc2pi")
            nc.sync.dma_start(out=fc2pi, in_=fc2pi_c.ap())
            coff = ptile([C, 1], dt.float32, "coff")
            nc.sync.dma_start(out=coff, in_=coff_c.ap())
            onesF = ptile([1, F], dt.bfloat16, "onesF")
            nc.vector.memset(onesF, 1.0)
            negpi = ptile([128, 1], dt.float32, "negpi")
            nc.vector.memset(negpi, -math.pi)
            onehot_f = ptile([128, 8, 8], dt.float32, "onehotf", pool=pp)
            nc.sync.dma_start(out=onehot_f, in_=onehot_c.ap())
            onehot = ptile([128, 8, 8], dt.bfloat16, "onehot")
            nc.vector.tensor_copy(onehot, onehot_f)
            zcol = ptile([128, 1], dt.float32, "zcol")
            nc.vector.memset(zcol, 0.0)

            def col(src_1d, nm):
                t = ptile([C, 1], dt.float32, nm)
                nc.sync.dma_start(out=t, in_=src_1d[:, None])
                return t

            ipbr_f = ptile([1, C], dt.float32, "ipbrf", pool=pp)
            nc.sync.dma_start(out=ipbr_f, in_=ipb_row)
            ipbr_bf = ptile([1, C], dt.bfloat16, "ipbrbf")
            nc.vector.tensor_copy(ipbr_bf, ipbr_f)
            conv_b_col = [col(conv_b[l], f"cb{l}") for l in range(L)]
            g_col = [col(ln_g[l], f"g{l}") for l in range(L)]
            b_col = [col(ln_b[l], f"b{l}") for l in range(L)]
            og_col = col(out_ln_g, "og")
            ob_col = col(out_ln_b, "ob")

            outb_row = ptile([1, DOUT], dt.float32, "outbr")
            nc.sync.dma_start(out=outb_row, in_=out_b[None, :])
            outb_bf = ptile([1, DOUT], dt.bfloat16, "outbbf")
            nc.vector.tensor_copy(outb_bf, outb_row)

            inw_f = ptile([128, 2, C], dt.float32, "inwf")
            nc.sync.dma_start(out=inw_f, in_=in_wT.rearrange("(a p) c -> p a c", p=128))
            inw_bf = ptile([128, 2, C], dt.bfloat16, "inwbf")
            nc.vector.tensor_copy(inw_bf, inw_f)
            posw_f = ptile([C, C], dt.float32, "poswf")
            nc.sync.dma_start(out=posw_f, in_=pos_wT)
            posw_bf = ptile([C, C], dt.bfloat16, "poswbf")
            nc.vector.tensor_copy(posw_bf, posw_f)
            outw_f = ptile([C, DOUT], dt.float32, "outwf")
            nc.sync.dma_start(out=outw_f, in_=out_wT)
            outw_bf = ptile([C, DOUT], dt.bfloat16, "outwbf")
            nc.vector.tensor_copy(outw_bf, outw_f)
            cw_bf = []
            for l in range(L):
                cwf = ptile([128, K, C], dt.float32, "cwstage", pool=pp, bufs=1)
                nc.sync.dma_start(out=cwf, in_=conv_wr[l].rearrange("k p c -> p k c"))
                cwb = ptile([128, K, C], dt.bfloat16, f"cw{l}")
                nc.vector.tensor_copy(cwb, cwf)
                cw_bf.append(cwb)

            # ---------- per-item persistent ----------
            xs, hs = [], []
            for b in range(BPC):
                xs.append(ptile([C, TP], dt.float32, f"x{b}"))
                h = ptile([C, HPAD + TP], dt.bfloat16, f"h{b}")
                nc.vector.memset(h[:, 0:HPAD], 0.0)
                hs.append(h)

            # ---------- cumsum ----------
            # cumd[0] = 0, cumd[1 + n] = cumsum(durations)[n]
            cumws, cumw0s = [], []
            def emit_cum(b):
                d32 = ptile([1, N], dt.int32, "d32", pool=pp)
                nc.sync.dma_start(out=d32, in_=durations[b][None, :])
                df = ptile([1, N], dt.float32, "df", pool=pp)
                nc.vector.tensor_copy(df, d32)
                zr = ptile([1, N], dt.float32, "zr", pool=pp)
                nc.vector.memset(zr, 0.0)
                cum = ptile([1, N], dt.float32, "cum", pool=pp)
                nc.vector.tensor_tensor_scan(cum, df, zr, 0.0, Alu.add, Alu.add)
                cumd = dram.tile([N + 1], dt.float32, tag="cumd", name="cumd", bufs=2)
                zel = pp.tile([1, 1], dt.float32, tag="zel", name="zel", bufs=2)
                nc.vector.memset(zel, 0.0)
                nc.sync.dma_start(out=cumd[None, 0:1], in_=zel)
                nc.sync.dma_start(out=cumd[None, 1:N + 1], in_=cum)
                cumw = ptile([128, NCH], dt.float32, f"cumw{b}", pool=pp)
                nc.sync.dma_start(out=cumw, in_=bass.AP(
                    tensor=cumd.tensor, offset=cumd.offset + 1, ap=[[1, 128], [128, NCH]]))
                cumw0 = ptile([128, NCH], dt.float32, f"cumw0{b}", pool=pp)
                nc.sync.dma_start(out=cumw0, in_=bass.AP(
                    tensor=cumd.tensor, offset=cumd.offset, ap=[[1, 128], [128, NCH]]))
                cumws.append(cumw)
                cumw0s.append(cumw0)

            # ---------- P'' projection (SBUF-resident, bf16) ----------
            psts = []
            def emit_proj(b):
                ptb = ptile([128, 2, N], dt.bfloat16, f"ptb{b}", pool=pp, bufs=1)
                for nchunk in range(NCH):
                    pch = pp.tile([128, DIN], dt.float32, tag="pch", name="pch", bufs=3)
                    nc.sync.dma_start(out=pch, in_=pooled[b, 128 * nchunk:128 * (nchunk + 1), :])
                    for dh in range(2):
                        tp = psum.tile([128, 128], dt.float32, tag="mmF", name="mmF", bufs=4)
                        nc.tensor.transpose(tp, pch[:, 128 * dh:128 * (dh + 1)], ident)
                        nc.vector.tensor_copy(
                            ptb[:, dh, 128 * nchunk:128 * (nchunk + 1)], tp)
                pst = ptile([128, NCH, C], dt.bfloat16, f"pst{b}")
                pstn = ptile([128, NCH, C], dt.bfloat16, f"pstn{b}")
                for nchunk in range(NCH):
                    pps = psum.tile([128, C], dt.float32, tag="mmF", name="mmF", bufs=4)
                    for dh in range(2):
                        nc.tensor.matmul(pps, ptb[:, dh, 128 * nchunk:128 * (nchunk + 1)],
                                         inw_bf[:, dh, :], start=(dh == 0), stop=(dh == 1))
                    nc.vector.tensor_copy(pst[:, nchunk, :], pps)
                    nc.vector.tensor_scalar(pstn[:, nchunk, :], pps, -1.0, None, Alu.mult)
                psts.append((pst, pstn))

            # ---------- upsample + posemb, one PSUM accumulation per slab ----
            # x[:, t] = ipb + sum_m P''[m] * D_m[t] + posw @ sin-emb[t], where
            # D_m[t] = [t >= cum[m-1]] - [t >= cum[m]] one-hots the phoneme m
            # active at frame t (idx is monotone, so masks replace the gather).
            def emit_upsample_slab(b, si):
                x = xs[b]
                sl = si * F
                w_val = max(0, min(F, T - sl))
                relb = pp.tile([128, F], dt.float32, tag="relb", name="relb", bufs=2)
                if w_val < F:
                    nc.vector.memset(relb, 0.0)
                if w_val > 0:
                    nc.gpsimd.dma_start(out=relb[:, 0:w_val], in_=bass.AP(
                        tensor=rel_pos.tensor,
                        offset=rel_pos.offset + b * T + sl,
                        ap=[[0, 128], [1, w_val]]))
                u = pp.tile([128, F], dt.float32, tag="u", name="u", bufs=2)
                nc.scalar.activation(u, relb, ActF.Identity, bias=coff, scale=fc2pi)
                k32 = pp.tile([128, F], dt.int32, tag="k32", name="k32", bufs=2)
                nc.vector.tensor_copy(k32, u)
                kf = pp.tile([128, F], dt.float32, tag="kf", name="kf", bufs=2)
                nc.vector.tensor_copy(kf, k32)
                nc.vector.tensor_tensor(u, u, kf, Alu.subtract)
                emb = pp.tile([128, F], dt.bfloat16, tag="emb", name="emb", bufs=2)
                nc.scalar.activation(emb, u, ActF.Sin, bias=zcol, scale=TWO_PI)
                bv1 = pp.tile([128, NCH], dt.float32, tag="bv1", name="bv1", bufs=2)
                nc.vector.tensor_scalar(bv1, cumws[b], 1.0, float(sl), Alu.mult, Alu.subtract)
                bv0 = pp.tile([128, NCH], dt.float32, tag="bv0", name="bv0", bufs=2)
                nc.vector.tensor_scalar(bv0, cumw0s[b], 1.0, float(sl), Alu.mult, Alu.subtract)
                pst, pstn = psts[b]
                xps = psum.tile([C, F], dt.float32, tag="mmF", name="mmF", bufs=4)
                for g in range(NCH):
                    # x += P''[m]*[t>=cum[m-1]] - P''[m]*[t>=cum[m]] : the two
                    # masks cancel exactly except on the band where phoneme m
                    # is active, so each frame nets exactly one P'' row.
                    S0 = pp.tile([128, F], dt.bfloat16, tag="S0", name="S0", bufs=4)
                    nc.vector.tensor_scalar(S0, iot, bv0[:, g:g + 1], None, Alu.is_ge)
                    nc.tensor.matmul(xps, pst[:, g, :], S0, start=(g == 0), stop=False)
                    S1 = pp.tile([128, F], dt.bfloat16, tag="S1", name="S1", bufs=4)
                    nc.vector.tensor_scalar(S1, iot, bv1[:, g:g + 1], None, Alu.is_ge)
                    nc.tensor.matmul(xps, pstn[:, g, :], S1, start=False, stop=False)
                nc.tensor.matmul(xps, ipbr_bf, onesF, start=False, stop=False)
                nc.tensor.matmul(xps, posw_bf, emb, start=False, stop=True)
                nc.scalar.copy(x[:, sl:sl + F], xps)

            # ---------- layernorm, slab-pipelined ----------
            # Stats as (8, F) PSUM rows (row si = slab si) via one-hot-column
            # ones-matmuls; row math on (8, F); per-frame scale/offset broadcast
            # back to (128, F) via rank-8 selector matmuls. All on-chip. The
            # emission interleaves each item's normalize+conv at slab level and
            # injects the other item's stats mid-stream so the PE never drains.
            def emit_stats_slab(b, S1, S2, si):
                x = xs[b]
                sl = si * F
                xbf = pp.tile([128, F], dt.bfloat16, tag="xbf", name="xbf", bufs=3)
                nc.scalar.activation(xbf, x[:, sl:sl + F], ActF.Copy)
                xsq = pp.tile([128, F], dt.bfloat16, tag="xsq", name="xsq", bufs=3)
                nc.scalar.activation(xsq, x[:, sl:sl + F], ActF.Square, bias=zcol)
                nc.tensor.matmul(S1, onehot[:, si, :], xbf,
                                 start=(si == 0), stop=(si == 7))
                nc.tensor.matmul(S2, onehot[:, si, :], xsq,
                                 start=(si == 0), stop=(si == 7))

            def emit_rowmath(S1, S2):
                mu = pp.tile([8, F], dt.float32, tag="mu", name="mu", bufs=1)
                nc.vector.tensor_scalar(mu, S1, 1.0 / C, None, Alu.mult)
                vr = pp.tile([8, F], dt.float32, tag="vr", name="vr", bufs=1)
                nc.vector.tensor_scalar(vr, S2, 1.0 / C, EPS, Alu.mult, Alu.add)
                mm2 = pp.tile([8, F], dt.float32, tag="mm2", name="mm2", bufs=1)
                nc.vector.tensor_tensor(mm2, mu, mu, Alu.mult)
                nc.vector.tensor_tensor(vr, vr, mm2, Alu.subtract)
                sq = pp.tile([8, F], dt.float32, tag="sq", name="sq", bufs=1)
                nc.scalar.activation(sq, vr, ActF.Sqrt)
                rF = pp.tile([8, F], dt.float32, tag="rF", name="rF", bufs=1)
                nc.vector.reciprocal_approx_fast(rF, sq)
                cF = pp.tile([8, F], dt.float32, tag="cF", name="cF", bufs=1)
                nc.vector.tensor_tensor(cF, mu, rF, Alu.mult)
                strc = dram.tile([2, 8, F], dt.float32, tag="strc", name="strc", bufs=2)
                nc.scalar.dma_start(out=strc[0], in_=rF)
                nc.scalar.dma_start(out=strc[1], in_=cF)
                return strc

            def emit_norm_slab(b, si, strc, gcol, bcol, out_act, out_tile, out_off):
                x = xs[b]
                sl = si * F
                AB = pp.tile([128, F], dt.float32, tag="ABs", name="ABs", bufs=3)
                nc.gpsimd.dma_start(out=AB, in_=bass.AP(
                    tensor=strc.tensor, offset=strc.offset + si * F,
                    ap=[[0, 128], [1, F]]))
                CB = pp.tile([128, F], dt.float32, tag="CBs", name="CBs", bufs=3)
                nc.sync.dma_start(out=CB, in_=bass.AP(
                    tensor=strc.tensor, offset=strc.offset + (8 + si) * F,
                    ap=[[0, 128], [1, F]]))
                t1 = pp.tile([128, F], dt.float32, tag="t1", name="t1", bufs=3)
                nc.vector.tensor_tensor(t1, x[:, sl:sl + F], AB, Alu.mult)
                nc.vector.tensor_tensor(t1, t1, CB, Alu.subtract)
                nc.scalar.activation(out_tile[:, out_off + sl:out_off + sl + F],
                                     t1, out_act, bias=bcol, scale=gcol)

            def emit_conv_slab(b, l, si):
                x, h = xs[b], hs[b]
                dil = dils[l]
                sl = si * F
                cv = psum.tile([128, F], dt.float32, tag="mmF", name="mmF", bufs=4)
                for k in range(K):
                    off = HPAD + sl - (K - 1 - k) * dil
                    nc.tensor.matmul(cv, cw_bf[l][:, k, :], h[:, off:off + F],
                                     start=(k == 0), stop=(k == K - 1))
                nc.vector.scalar_tensor_tensor(
                    x[:, sl:sl + F], cv, conv_b_col[l], x[:, sl:sl + F],
                    Alu.add, Alu.add)

            def emit_out_chunk(b, tchunk):
                ybf = hs[b]
                t0 = 128 * tchunk
                nrows = min(128, T - t0)
                if nrows <= 0:
                    return
                po = psum.tile([128, DOUT], dt.float32, tag="mmF", name="mmF", bufs=4)
                nc.tensor.matmul(po, ybf[:, HPAD + t0:HPAD + t0 + 128], outw_bf,
                                 start=True, stop=False)
                nc.tensor.matmul(po, onesF[:, 0:128], outb_bf, start=False, stop=True)
                ost = pp.tile([128, DOUT], dt.float32, tag="ost", name="ost", bufs=3)
                if tchunk % 2 == 0:
                    nc.scalar.copy(ost, po)
                else:
                    nc.vector.tensor_copy(ost, po)
                nc.sync.dma_start(out=out[b, t0:t0 + nrows, :], in_=ost[:nrows, :])

            for b in range(BPC):
                emit_cum(b)
            for b in range(BPC):
                emit_proj(b)

            # pipeline: (b, stage) steps; stage 0..L-1 = conv layers, L = out
            SEQ = [(b_, s_) for s_ in range(L + 1) for b_ in range(BPC)]
            # out chunk tchunk is ready once norm slab owner_slab[tchunk] is done
            owner_slab = [min(7, ((tc_ + 1) * 128 - 1) // F) for tc_ in range(NT)]
            stats_tiles = {}

            def emit_stats_all(b, start_si, end_si):
                if start_si == 0:
                    stats_tiles[b] = (
                        psum.tile([8, F], dt.float32, tag="st", name="st", bufs=4),
                        psum.tile([8, F], dt.float32, tag="st", name="st", bufs=4))
                S1, S2 = stats_tiles[b]
                for si in range(start_si, end_si):
                    emit_stats_slab(b, S1, S2, si)

            rcs = {}
            for si in range(8):
                for b in range(BPC):
                    emit_upsample_slab(b, si)
                    emit_stats_all(b, si, si + 1)
                    if si == 7 and b == 0:
                        rcs[0] = emit_rowmath(*stats_tiles[0])
            for step, (b, stage) in enumerate(SEQ):
                nxt = SEQ[step + 1] if step + 1 < len(SEQ) else None
                if stage < L:
                    gcol, bcol, act, otile, ooff = (
                        g_col[stage], b_col[stage], _GELU, hs[b], HPAD)
                else:
                    gcol, bcol, act, otile, ooff = (
                        og_col, ob_col, ActF.Identity, hs[b], HPAD)
                oc = 0
                for si in range(8):
                    emit_norm_slab(b, si, rcs[b], gcol, bcol, act, otile, ooff)
                    if stage < L:
                        emit_conv_slab(b, stage, si)
                    else:
                        while oc < NT and owner_slab[oc] <= si:
                            emit_out_chunk(b, oc)
                            oc += 1
                    # inject next step's stats (2 slabs each at si=2..5),
                    # rowmath after si==5 so it overlaps slabs 6-7 here
                    if nxt is not None:
                        if 2 <= si <= 5 and step > 0:
                            emit_stats_all(nxt[0], (si - 2) * 2, (si - 2) * 2 + 2)
                        elif si == 6:
                            rcs[nxt[0]] = emit_rowmath(*stats_tiles[nxt[0]])

    nc.compile()
    return nc


_NC_CACHE = {}


def _get_nc(T):
    if T not in _NC_CACHE:
        _NC_CACHE[T] = build_nc(T)
    return _NC_CACHE[T]


def make_in_maps(pooled, rel_pos, in_w, in_b, pos_w, pos_b, conv_w, conv_b,
                 ln_g, ln_b, out_ln_g, out_ln_b, out_w, out_b, durations):
    shared = {
        "in_wT": np.ascontiguousarray(np.asarray(in_w, np.float32).T),
        "in_b": np.asarray(in_b, np.float32),
        "pos_wT": np.ascontiguousarray(np.asarray(pos_w, np.float32).T),
        "pos_b": np.asarray(pos_b, np.float32),
        "conv_wr": np.ascontiguousarray(np.asarray(conv_w, np.float32).transpose(0, 3, 2, 1)),
        "conv_b": np.asarray(conv_b, np.float32),
        "ln_g": np.asarray(ln_g, np.float32),
        "ln_b": np.asarray(ln_b, np.float32),
        "out_ln_g": np.asarray(out_ln_g, np.float32),
        "out_ln_b": np.asarray(out_ln_b, np.float32),
        "out_wT": np.ascontiguousarray(np.asarray(out_w, np.float32).T),
        "out_b": np.asarray(out_b, np.float32),
        "ipb_row": (np.asarray(in_b, np.float32)
                    + np.asarray(pos_b, np.float32))[None, :],
    }
    in_maps = []
    for c in range(NCORES):
        s = slice(c * BPC, (c + 1) * BPC)
        m = dict(shared)
        m["pooled"] = np.ascontiguousarray(np.asarray(pooled, np.float32)[s])
        m["durations"] = np.ascontiguousarray(np.asarray(durations, np.int32)[s])
        m["rel_pos"] = np.ascontiguousarray(np.asarray(rel_pos, np.float32)[s])
        in_maps.append(m)
    return in_maps


def kernel(**inputs):
    T = inputs["rel_pos"].shape[1]
    nc = _get_nc(T)
    in_maps = make_in_maps(**inputs)
    res = bass_utils.run_bass_kernel_spmd(nc, in_maps, core_ids=list(range(NCORES)))
    return np.concatenate([res.results[c]["out"] for c in range(NCORES)],
                          axis=0).astype(np.float32)



# revision 38
# speedup vs baseline: 1.0341x; 1.0341x over previous
"""DurationConditioningProjector Trainium2 kernel.

Data-parallel over batch B=16 across 8 NeuronCores (2 items per core).
Everything is computed on-device; the host only slices/replicates inputs
(pure relayout of weight matrices) and reassembles the output.

Per-item layout: residual x as (C=128 partitions, T free) fp32 in SBUF.
- Length-regulator upsample AS MATMUL: idx is monotone (cumsum of
  durations), so x[:,t] = P''[idx[t]] is computed as paired band-mask
  matmuls  sum_m P''[m]*[t>=cum[m-1]] - P''[m]*[t>=cum[m]]  where the
  {0,1} masks come from DVE is_ge tensor_scalar ops on an iota tile.
  The two mask terms cancel exactly off-band, so each frame nets exactly
  one bf16 P'' row (gather-grade precision; no dma_gather, no DRAM
  index/table round-trips). in_b+pos_b ride in as a rank-1 ones matmul;
  the sin/cos pos-emb matmul accumulates into the same PSUM bank.
- LayerNorm stays on-chip: per-slab channel sums/sumsq accumulate into
  (8,F) PSUM rows via one-hot-column matmuls; row math (mean/var, rsqrt
  via Sqrt + reciprocal_approx_fast) on (8,F) tiles; per-frame scale and
  offset broadcast to (128,F) by 0-stride DMA from a small DRAM bounce;
  normalize = 2 DVE TTs + fused ACT gelu(g*z+b).
- 3 dilated causal conv layers: 31 shifted bf16 matmuls per slab into
  PSUM; residual add is one fused scalar_tensor_tensor. Emission is
  software-pipelined at slab granularity (norm slab -> conv slab, other
  item's stats injected mid-stream) so the PE never drains and HAM
  stays warm. Output LN + per-chunk matmuls with out_b folded in as a
  rank-1 accumulate.
"""
import sys
sys.path.insert(0, '/opt/trn_rl_repo')

import math
import os
import numpy as np

import concourse.bass as bass
import concourse.mybir as mybir
import concourse.tile as tile
from concourse import bacc
from concourse import bass_utils

dt = mybir.dt
Alu = mybir.AluOpType
ActF = mybir.ActivationFunctionType
_GELU = ActF.Tanh if os.environ.get('KSIM_TANH') else ActF.Gelu

B, N, DIN, C, DOUT, K, L = 16, 1024, 256, 128, 256, 31, 3
NCORES = 8
BPC = B // NCORES
TWO_PI = 2.0 * math.pi
EPS = 1e-5


def _ceil_to(x, m):
    return (x + m - 1) // m * m


def build_nc(T):
    TP = _ceil_to(T, 128)
    NT = TP // 128
    F = TP // 8
    assert F % 16 == 0 and F <= 512
    NCH = N // 128
    NGC = (TP + 511) // 512       # gather chunks

    nc = bacc.Bacc("TRN2", target_bir_lowering=False, debug=False)

    pooled = nc.dram_tensor("pooled", [BPC, N, DIN], dt.float32, kind="ExternalInput").ap()
    durations = nc.dram_tensor("durations", [BPC, N], dt.int32, kind="ExternalInput").ap()
    rel_pos = nc.dram_tensor("rel_pos", [BPC, T], dt.float32, kind="ExternalInput").ap()
    in_wT = nc.dram_tensor("in_wT", [DIN, C], dt.float32, kind="ExternalInput").ap()
    in_b = nc.dram_tensor("in_b", [C], dt.float32, kind="ExternalInput").ap()
    pos_wT = nc.dram_tensor("pos_wT", [C, C], dt.float32, kind="ExternalInput").ap()
    pos_b = nc.dram_tensor("pos_b", [C], dt.float32, kind="ExternalInput").ap()
    conv_wr = nc.dram_tensor("conv_wr", [L, K, C, C], dt.float32, kind="ExternalInput").ap()
    conv_b = nc.dram_tensor("conv_b", [L, C], dt.float32, kind="ExternalInput").ap()
    ln_g = nc.dram_tensor("ln_g", [L, C], dt.float32, kind="ExternalInput").ap()
    ln_b = nc.dram_tensor("ln_b", [L, C], dt.float32, kind="ExternalInput").ap()
    out_ln_g = nc.dram_tensor("out_ln_g", [C], dt.float32, kind="ExternalInput").ap()
    out_ln_b = nc.dram_tensor("out_ln_b", [C], dt.float32, kind="ExternalInput").ap()
    out_wT = nc.dram_tensor("out_wT", [C, DOUT], dt.float32, kind="ExternalInput").ap()
    out_b = nc.dram_tensor("out_b", [DOUT], dt.float32, kind="ExternalInput").ap()
    ipb_row = nc.dram_tensor("ipb_row", [1, C], dt.float32, kind="ExternalInput").ap()
    out = nc.dram_tensor("out", [BPC, T, DOUT], dt.float32, kind="ExternalOutput").ap()

    ident_c = nc.inline_tensor(np.eye(128, dtype=np.float32), "identc")
    iota_c = nc.inline_tensor(
        np.broadcast_to(np.arange(F, dtype=np.float32), (128, F)).copy(), "iotac")
    # onehot_c[p, si, j] = 1 iff j == si : lhsT column-selector for stats rows
    oh = np.zeros((8, 8), np.float32)
    np.fill_diagonal(oh, 1.0)
    onehot_c = nc.inline_tensor(
        np.broadcast_to(oh[None, :, :], (128, 8, 8)).copy(), "onehotc")
    half = C // 2
    freqs = np.exp(-math.log(10000.0) * np.arange(half, dtype=np.float64) / (half - 1))
    fc2pi_c = nc.inline_tensor(
        (np.concatenate([freqs, freqs]) / TWO_PI).astype(np.float32)[:, None], "fc2pic")
    coff_np = np.zeros((C, 1), np.float32)
    coff_np[half:] = 0.25
    coff_c = nc.inline_tensor(coff_np, "coffc")

    dils = [2 ** i for i in range(L)]
    HPAD = (K - 1) * dils[-1]

    with tile.TileContext(nc) as tc:
        import contextlib
        ctx = contextlib.ExitStack()
        with ctx:
            pers = ctx.enter_context(tc.tile_pool(name="pers", bufs=1))
            pp = ctx.enter_context(tc.tile_pool(name="pp", bufs=2))
            psum = ctx.enter_context(tc.tile_pool(name="ps", bufs=1, space="PSUM"))
            dram = ctx.enter_context(tc.tile_pool(name="dr", bufs=1, space="DRAM"))

            def ptile(shape, d, nm, bufs=1, pool=None):
                pool = pool or pers
                return pool.tile(shape, d, tag=nm, name=nm, bufs=bufs)

            # ---------- shared setup ----------
            ident = ptile([128, 128], dt.float32, "ident")
            nc.sync.dma_start(out=ident, in_=ident_c.ap())
            iot = ptile([128, F], dt.float32, "iot")
            nc.sync.dma_start(out=iot, in_=iota_c.ap())
            fc2pi = ptile([C, 1], dt.float32, "fc2pi")
            nc.sync.dma_start(out=fc2pi, in_=fc2pi_c.ap())
            coff = ptile([C, 1], dt.float32, "coff")
            nc.sync.dma_start(out=coff, in_=coff_c.ap())
            onesF = ptile([1, F], dt.bfloat16, "onesF")
            nc.vector.memset(onesF, 1.0)
            negpi = ptile([128, 1], dt.float32, "negpi")
            nc.vector.memset(negpi, -math.pi)
            onehot_f = ptile([128, 8, 8], dt.float32, "onehotf", pool=pp)
            nc.sync.dma_start(out=onehot_f, in_=onehot_c.ap())
            onehot = ptile([128, 8, 8], dt.bfloat16, "onehot")
            nc.vector.tensor_copy(onehot, onehot_f)
            zcol = ptile([128, 1], dt.float32, "zcol")
            nc.vector.memset(zcol, 0.0)

            def col(src_1d, nm):
                t = ptile([C, 1], dt.float32, nm)
                nc.sync.dma_start(out=t, in_=src_1d[:, None])
                return t

            ipbr_f = ptile([1, C], dt.float32, "ipbrf", pool=pp)
            nc.sync.dma_start(out=ipbr_f, in_=ipb_row)
            ipbr_bf = ptile([1, C], dt.bfloat16, "ipbrbf")
            nc.vector.tensor_copy(ipbr_bf, ipbr_f)
            conv_b_col = [col(conv_b[l], f"cb{l}") for l in range(L)]
            g_col = [col(ln_g[l], f"g{l}") for l in range(L)]
            b_col = [col(ln_b[l], f"b{l}") for l in range(L)]
            og_col = col(out_ln_g, "og")
            ob_col = col(out_ln_b, "ob")

            outb_row = ptile([1, DOUT], dt.float32, "outbr")
            nc.sync.dma_start(out=outb_row, in_=out_b[None, :])
            outb_bf = ptile([1, DOUT], dt.bfloat16, "outbbf")
            nc.vector.tensor_copy(outb_bf, outb_row)

            inw_f = ptile([128, 2, C], dt.float32, "inwf")
            nc.sync.dma_start(out=inw_f, in_=in_wT.rearrange("(a p) c -> p a c", p=128))
            inw_bf = ptile([128, 2, C], dt.bfloat16, "inwbf")
            nc.vector.tensor_copy(inw_bf, inw_f)
            posw_f = ptile([C, C], dt.float32, "poswf")
            nc.sync.dma_start(out=posw_f, in_=pos_wT)
            posw_bf = ptile([C, C], dt.bfloat16, "poswbf")
            nc.vector.tensor_copy(posw_bf, posw_f)
            outw_f = ptile([C, DOUT], dt.float32, "outwf")
            nc.sync.dma_start(out=outw_f, in_=out_wT)
            outw_bf = ptile([C, DOUT], dt.bfloat16, "outwbf")
            nc.vector.tensor_copy(outw_bf, outw_f)
            cw_bf = []
            for l in range(L):
                cwf = ptile([128, K, C], dt.float32, "cwstage", pool=pp, bufs=1)
                nc.sync.dma_start(out=cwf, in_=conv_wr[l].rearrange("k p c -> p k c"))
                cwb = ptile([128, K, C], dt.bfloat16, f"cw{l}")
                nc.vector.tensor_copy(cwb, cwf)
                cw_bf.append(cwb)

            # ---------- per-item persistent ----------
            xs, hs = [], []
            for b in range(BPC):
                xs.append(ptile([C, TP], dt.float32, f"x{b}"))
                h = ptile([C, HPAD + TP], dt.bfloat16, f"h{b}")
                nc.vector.memset(h[:, 0:HPAD], 0.0)
                hs.append(h)

            # ---------- cumsum ----------
            # cumd[0] = 0, cumd[1 + n] = cumsum(durations)[n]
            cumws, cumw0s = [], []
            def emit_cum(b):
                d32 = ptile([1, N], dt.int32, "d32", pool=pp)
                nc.sync.dma_start(out=d32, in_=durations[b][None, :])
                df = ptile([1, N], dt.float32, "df", pool=pp)
                nc.vector.tensor_copy(df, d32)
                zr = ptile([1, N], dt.float32, "zr", pool=pp)
                nc.vector.memset(zr, 0.0)
                cum = ptile([1, N], dt.float32, "cum", pool=pp)
                nc.vector.tensor_tensor_scan(cum, df, zr, 0.0, Alu.add, Alu.add)
                cumd = dram.tile([N + 1], dt.float32, tag="cumd", name="cumd", bufs=2)
                zel = pp.tile([1, 1], dt.float32, tag="zel", name="zel", bufs=2)
                nc.vector.memset(zel, 0.0)
                nc.sync.dma_start(out=cumd[None, 0:1], in_=zel)
                nc.sync.dma_start(out=cumd[None, 1:N + 1], in_=cum)
                cumw = ptile([128, NCH], dt.float32, f"cumw{b}", pool=pp)
                nc.sync.dma_start(out=cumw, in_=bass.AP(
                    tensor=cumd.tensor, offset=cumd.offset + 1, ap=[[1, 128], [128, NCH]]))
                cumw0 = ptile([128, NCH], dt.float32, f"cumw0{b}", pool=pp)
                nc.sync.dma_start(out=cumw0, in_=bass.AP(
                    tensor=cumd.tensor, offset=cumd.offset, ap=[[1, 128], [128, NCH]]))
                cumws.append(cumw)
                cumw0s.append(cumw0)

            # ---------- P'' projection (SBUF-resident, bf16) ----------
            psts = []
            def emit_proj(b):
                ptb = ptile([128, 2, N], dt.bfloat16, f"ptb{b}", pool=pp, bufs=1)
                for nchunk in range(NCH):
                    pch = pp.tile([128, DIN], dt.float32, tag="pch", name="pch", bufs=3)
                    nc.sync.dma_start(out=pch, in_=pooled[b, 128 * nchunk:128 * (nchunk + 1), :])
                    for dh in range(2):
                        tp = psum.tile([128, 128], dt.float32, tag="mmF", name="mmF", bufs=4)
                        nc.tensor.transpose(tp, pch[:, 128 * dh:128 * (dh + 1)], ident)
                        nc.vector.tensor_copy(
                            ptb[:, dh, 128 * nchunk:128 * (nchunk + 1)], tp)
                pst = ptile([128, NCH, C], dt.bfloat16, f"pst{b}")
                pstn = ptile([128, NCH, C], dt.bfloat16, f"pstn{b}")
                for nchunk in range(NCH):
                    pps = psum.tile([128, C], dt.float32, tag="mmF", name="mmF", bufs=4)
                    for dh in range(2):
                        nc.tensor.matmul(pps, ptb[:, dh, 128 * nchunk:128 * (nchunk + 1)],
                                         inw_bf[:, dh, :], start=(dh == 0), stop=(dh == 1))
                    nc.vector.tensor_copy(pst[:, nchunk, :], pps)
                    nc.vector.tensor_scalar(pstn[:, nchunk, :], pps, -1.0, None, Alu.mult)
                psts.append((pst, pstn))

            # ---------- upsample + posemb, one PSUM accumulation per slab ----
            # x[:, t] = ipb + sum_m P''[m] * D_m[t] + posw @ sin-emb[t], where
            # D_m[t] = [t >= cum[m-1]] - [t >= cum[m]] one-hots the phoneme m
            # active at frame t (idx is monotone, so masks replace the gather).
            def emit_upsample_slab(b, si):
                x = xs[b]
                sl = si * F
                w_val = max(0, min(F, T - sl))
                relb = pp.tile([128, F], dt.float32, tag="relb", name="relb", bufs=2)
                if w_val < F:
                    nc.vector.memset(relb, 0.0)
                if w_val > 0:
                    nc.gpsimd.dma_start(out=relb[:, 0:w_val], in_=bass.AP(
                        tensor=rel_pos.tensor,
                        offset=rel_pos.offset + b * T + sl,
                        ap=[[0, 128], [1, w_val]]))
                u = pp.tile([128, F], dt.float32, tag="u", name="u", bufs=2)
                nc.scalar.activation(u, relb, ActF.Identity, bias=coff, scale=fc2pi)
                k32 = pp.tile([128, F], dt.int32, tag="k32", name="k32", bufs=2)
                nc.vector.tensor_copy(k32, u)
                kf = pp.tile([128, F], dt.float32, tag="kf", name="kf", bufs=2)
                nc.vector.tensor_copy(kf, k32)
                nc.vector.tensor_tensor(u, u, kf, Alu.subtract)
                emb = pp.tile([128, F], dt.bfloat16, tag="emb", name="emb", bufs=2)
                nc.scalar.activation(emb, u, ActF.Sin, bias=zcol, scale=TWO_PI)
                bv1 = pp.tile([128, NCH], dt.float32, tag="bv1", name="bv1", bufs=2)
                nc.vector.tensor_scalar(bv1, cumws[b], 1.0, float(sl), Alu.mult, Alu.subtract)
                bv0 = pp.tile([128, NCH], dt.float32, tag="bv0", name="bv0", bufs=2)
                nc.vector.tensor_scalar(bv0, cumw0s[b], 1.0, float(sl), Alu.mult, Alu.subtract)
                pst, pstn = psts[b]
                xps = psum.tile([C, F], dt.float32, tag="mmF", name="mmF", bufs=4)
                # durations < DUR_MAX=8, so cum[m] <= 8*(m+1): chunk g cannot
                # intersect this slab when 8*128*(g+1) <= sl - skip it.
                g0 = 0
                while 1024 * (g0 + 1) <= sl:
                    g0 += 1
                first = True
                for g in range(g0, NCH):
                    # x += P''[m]*[t>=cum[m-1]] - P''[m]*[t>=cum[m]] : the two
                    # masks cancel exactly except on the band where phoneme m
                    # is active, so each frame nets exactly one P'' row.
                    S0 = pp.tile([128, F], dt.bfloat16, tag="S0", name="S0", bufs=4)
                    nc.vector.tensor_scalar(S0, iot, bv0[:, g:g + 1], None, Alu.is_ge)
                    nc.tensor.matmul(xps, pst[:, g, :], S0, start=first, stop=False)
                    first = False
                    S1 = pp.tile([128, F], dt.bfloat16, tag="S1", name="S1", bufs=4)
                    nc.vector.tensor_scalar(S1, iot, bv1[:, g:g + 1], None, Alu.is_ge)
                    nc.tensor.matmul(xps, pstn[:, g, :], S1, start=False, stop=False)
                nc.tensor.matmul(xps, ipbr_bf, onesF, start=False, stop=False)
                nc.tensor.matmul(xps, posw_bf, emb, start=False, stop=True)
                nc.scalar.copy(x[:, sl:sl + F], xps)

            # ---------- layernorm, slab-pipelined ----------
            # Stats as (8, F) PSUM rows (row si = slab si) via one-hot-column
            # ones-matmuls; row math on (8, F); per-frame scale/offset broadcast
            # back to (128, F) via rank-8 selector matmuls. All on-chip. The
            # emission interleaves each item's normalize+conv at slab level and
            # injects the other item's stats mid-stream so the PE never drains.
            def emit_stats_slab(b, S1, S2, si):
                x = xs[b]
                sl = si * F
                xbf = pp.tile([128, F], dt.bfloat16, tag="xbf", name="xbf", bufs=3)
                nc.scalar.activation(xbf, x[:, sl:sl + F], ActF.Copy)
                xsq = pp.tile([128, F], dt.bfloat16, tag="xsq", name="xsq", bufs=3)
                nc.scalar.activation(xsq, x[:, sl:sl + F], ActF.Square, bias=zcol)
                nc.tensor.matmul(S1, onehot[:, si, :], xbf,
                                 start=(si == 0), stop=(si == 7))
                nc.tensor.matmul(S2, onehot[:, si, :], xsq,
                                 start=(si == 0), stop=(si == 7))

            def emit_rowmath(S1, S2):
                mu = pp.tile([8, F], dt.float32, tag="mu", name="mu", bufs=1)
                nc.vector.tensor_scalar(mu, S1, 1.0 / C, None, Alu.mult)
                vr = pp.tile([8, F], dt.float32, tag="vr", name="vr", bufs=1)
                nc.vector.tensor_scalar(vr, S2, 1.0 / C, EPS, Alu.mult, Alu.add)
                mm2 = pp.tile([8, F], dt.float32, tag="mm2", name="mm2", bufs=1)
                nc.vector.tensor_tensor(mm2, mu, mu, Alu.mult)
                nc.vector.tensor_tensor(vr, vr, mm2, Alu.subtract)
                sq = pp.tile([8, F], dt.float32, tag="sq", name="sq", bufs=1)
                nc.scalar.activation(sq, vr, ActF.Sqrt)
                rF = pp.tile([8, F], dt.float32, tag="rF", name="rF", bufs=1)
                nc.vector.reciprocal_approx_fast(rF, sq)
                cF = pp.tile([8, F], dt.float32, tag="cF", name="cF", bufs=1)
                nc.vector.tensor_tensor(cF, mu, rF, Alu.mult)
                strc = dram.tile([2, 8, F], dt.float32, tag="strc", name="strc", bufs=2)
                nc.scalar.dma_start(out=strc[0], in_=rF)
                nc.scalar.dma_start(out=strc[1], in_=cF)
                return strc

            def emit_norm_slab(b, si, strc, gcol, bcol, out_act, out_tile, out_off):
                x = xs[b]
                sl = si * F
                AB = pp.tile([128, F], dt.float32, tag="ABs", name="ABs", bufs=3)
                nc.gpsimd.dma_start(out=AB, in_=bass.AP(
                    tensor=strc.tensor, offset=strc.offset + si * F,
                    ap=[[0, 128], [1, F]]))
                CB = pp.tile([128, F], dt.float32, tag="CBs", name="CBs", bufs=3)
                nc.sync.dma_start(out=CB, in_=bass.AP(
                    tensor=strc.tensor, offset=strc.offset + (8 + si) * F,
                    ap=[[0, 128], [1, F]]))
                t1 = pp.tile([128, F], dt.float32, tag="t1", name="t1", bufs=3)
                nc.vector.tensor_tensor(t1, x[:, sl:sl + F], AB, Alu.mult)
                nc.vector.tensor_tensor(t1, t1, CB, Alu.subtract)
                nc.scalar.activation(out_tile[:, out_off + sl:out_off + sl + F],
                                     t1, out_act, bias=bcol, scale=gcol)

            def emit_conv_slab(b, l, si):
                x, h = xs[b], hs[b]
                dil = dils[l]
                sl = si * F
                cv = psum.tile([128, F], dt.float32, tag="mmF", name="mmF", bufs=4)
                for k in range(K):
                    off = HPAD + sl - (K - 1 - k) * dil
                    nc.tensor.matmul(cv, cw_bf[l][:, k, :], h[:, off:off + F],
                                     start=(k == 0), stop=(k == K - 1))
                nc.vector.scalar_tensor_tensor(
                    x[:, sl:sl + F], cv, conv_b_col[l], x[:, sl:sl + F],
                    Alu.add, Alu.add)

            def emit_out_chunk(b, tchunk):
                ybf = hs[b]
                t0 = 128 * tchunk
                nrows = min(128, T - t0)
                if nrows <= 0:
                    return
                po = psum.tile([128, DOUT], dt.float32, tag="mmF", name="mmF", bufs=4)
                nc.tensor.matmul(po, ybf[:, HPAD + t0:HPAD + t0 + 128], outw_bf,
                                 start=True, stop=False)
                nc.tensor.matmul(po, onesF[:, 0:128], outb_bf, start=False, stop=True)
                ost = pp.tile([128, DOUT], dt.float32, tag="ost", name="ost", bufs=3)
                if tchunk % 2 == 0:
                    nc.scalar.copy(ost, po)
                else:
                    nc.vector.tensor_copy(ost, po)
                nc.sync.dma_start(out=out[b, t0:t0 + nrows, :], in_=ost[:nrows, :])

            for b in range(BPC):
                emit_cum(b)
            for b in range(BPC):
                emit_proj(b)

            # pipeline: (b, stage) steps; stage 0..L-1 = conv layers, L = out
            SEQ = [(b_, s_) for s_ in range(L + 1) for b_ in range(BPC)]
            # out chunk tchunk is ready once norm slab owner_slab[tchunk] is done
            owner_slab = [min(7, ((tc_ + 1) * 128 - 1) // F) for tc_ in range(NT)]
            stats_tiles = {}

            def emit_stats_all(b, start_si, end_si):
                if start_si == 0:
                    stats_tiles[b] = (
                        psum.tile([8, F], dt.float32, tag="st", name="st", bufs=4),
                        psum.tile([8, F], dt.float32, tag="st", name="st", bufs=4))
                S1, S2 = stats_tiles[b]
                for si in range(start_si, end_si):
                    emit_stats_slab(b, S1, S2, si)

            rcs = {}
            for si in range(8):
                for b in range(BPC):
                    emit_upsample_slab(b, si)
                    emit_stats_all(b, si, si + 1)
                    if si == 7 and b == 0:
                        rcs[0] = emit_rowmath(*stats_tiles[0])
            for step, (b, stage) in enumerate(SEQ):
                nxt = SEQ[step + 1] if step + 1 < len(SEQ) else None
                if stage < L:
                    gcol, bcol, act, otile, ooff = (
                        g_col[stage], b_col[stage], _GELU, hs[b], HPAD)
                else:
                    gcol, bcol, act, otile, ooff = (
                        og_col, ob_col, ActF.Identity, hs[b], HPAD)
                oc = 0
                for si in range(8):
                    emit_norm_slab(b, si, rcs[b], gcol, bcol, act, otile, ooff)
                    if stage < L:
                        emit_conv_slab(b, stage, si)
                    else:
                        while oc < NT and owner_slab[oc] <= si:
                            emit_out_chunk(b, oc)
                            oc += 1
                    # inject next step's stats (2 slabs each at si=2..5),
                    # rowmath after si==5 so it overlaps slabs 6-7 here
                    if nxt is not None:
                        if 2 <= si <= 5 and step > 0:
                            emit_stats_all(nxt[0], (si - 2) * 2, (si - 2) * 2 + 2)
                        elif si == 6:
                            rcs[nxt[0]] = emit_rowmath(*stats_tiles[nxt[0]])

    nc.compile()
    return nc


_NC_CACHE = {}


def _get_nc(T):
    if T not in _NC_CACHE:
        _NC_CACHE[T] = build_nc(T)
    return _NC_CACHE[T]


def make_in_maps(pooled, rel_pos, in_w, in_b, pos_w, pos_b, conv_w, conv_b,
                 ln_g, ln_b, out_ln_g, out_ln_b, out_w, out_b, durations):
    shared = {
        "in_wT": np.ascontiguousarray(np.asarray(in_w, np.float32).T),
        "in_b": np.asarray(in_b, np.float32),
        "pos_wT": np.ascontiguousarray(np.asarray(pos_w, np.float32).T),
        "pos_b": np.asarray(pos_b, np.float32),
        "conv_wr": np.ascontiguousarray(np.asarray(conv_w, np.float32).transpose(0, 3, 2, 1)),
        "conv_b": np.asarray(conv_b, np.float32),
        "ln_g": np.asarray(ln_g, np.float32),
        "ln_b": np.asarray(ln_b, np.float32),
        "out_ln_g": np.asarray(out_ln_g, np.float32),
        "out_ln_b": np.asarray(out_ln_b, np.float32),
        "out_wT": np.ascontiguousarray(np.asarray(out_w, np.float32).T),
        "out_b": np.asarray(out_b, np.float32),
        "ipb_row": (np.asarray(in_b, np.float32)
                    + np.asarray(pos_b, np.float32))[None, :],
    }
    in_maps = []
    for c in range(NCORES):
        s = slice(c * BPC, (c + 1) * BPC)
        m = dict(shared)
        m["pooled"] = np.ascontiguousarray(np.asarray(pooled, np.float32)[s])
        m["durations"] = np.ascontiguousarray(np.asarray(durations, np.int32)[s])
        m["rel_pos"] = np.ascontiguousarray(np.asarray(rel_pos, np.float32)[s])
        in_maps.append(m)
    return in_maps


def kernel(**inputs):
    T = inputs["rel_pos"].shape[1]
    nc = _get_nc(T)
    in_maps = make_in_maps(**inputs)
    res = bass_utils.run_bass_kernel_spmd(nc, in_maps, core_ids=list(range(NCORES)))
    return np.concatenate([res.results[c]["out"] for c in range(NCORES)],
                          axis=0).astype(np.float32)

